# revision 11
# baseline (speedup 1.0000x reference)
"""Masked multi-head attention kernel for Trainium2 (Bass/Tile), 8-core SPMD.

Problem: BH=64 heads of S=2048, D=64 attention with a dense bool mask,
scale = 1/sqrt(1024).  Sharded 8 heads per NeuronCore (no cross-core comm).

Per-core dataflow (heads processed in pairs; ACT-exp paces the kernel):
  - Q,K: SWDGE cast-DMA f32->bf16 HBM->HBM into a pair-interleaved scratch
    [S, 128] (head A cols 0:64, head B cols 64:128), then one HWDGE xbar
    transpose-DMA per (pair, tensor) builds the [d2, S] slab in SBUF.
  - V: SWDGE cast-DMA into [128, 16*65] chunk-major tiles; col 64 of each
    65-group memset to 1.0 (gives softmax denominators via the AV matmul).
  - mask: host sends KEEP mask (1=keep) transposed [H, S_k, S_q] u8; SWDGE
    cast-DMA u8->bf16 into quarter slabs laid out [p, qc, kl, j] so the
    multiply operand per (qc, 4-chunk group) is one contiguous [128, 2048].
  - S^T = K Q^T on the PE with the two heads' matmuls issued adjacently
    (disjoint 64-row groups -> concurrent); scores land in PSUM as bf16 so
    a [128, 2048] tile is 2 banks and exp runs at FD=2048.
  - exp on ACT (scale=1/32) PSUM->SBUF bf16; mask applied post-exp as one
    DVE tensor_mul [128, 2048] (bf16 2x mode) giving exact zeros.
  - AV: stationary [V|1] (M=65) streams masked P^T, accumulating the 8
    k-chunks of a half in PSUM; halves combined with one DVE add.
  - Epilogue per (qc, head): 4 PE transposes into one packed PSUM tile,
    one strided reciprocal, 4 per-partition scales, one batched store.
  - Cross-pair prefetch: next pair's slab/V DMAs are emitted between the
    current pair's early and late mask quarters so they are not FIFO-stuck
    behind 16 MB of mask traffic on the single SWDGE queue.
"""

import os
import sys

sys.path.insert(0, "/opt/trn_rl_repo")

import numpy as np

import concourse.bass as bass
import concourse.mybir as mybir
import concourse.tile as tile
from concourse import bacc
from concourse.bass_utils import run_bass_kernel_spmd
from concourse.masks import make_identity

N_CORES = 8
BH, S_FULL, D = 64, 2048, 64
H_PER_CORE = BH // N_CORES  # 8
P = 128  # SBUF/PSUM partitions
KCH = 128  # k-chunk rows
QCH = 512  # q-chunk cols
SCALE = 1.0 / 32.0  # 1/sqrt(1024) per the module spec


def build_attention(tc, o_ap, q_ap, k_ap, v_ap, m_ap, H, S):
    nc = tc.nc
    dt = mybir.dt
    n_pairs = H // 2
    n_kch = S // KCH  # 16 k-chunks per head
    n_qc = S // QCH  # 4 q-chunks
    n_quart = 4
    KLQ = n_kch // n_quart  # 4 k-chunks per mask quarter
    QW = KLQ * QCH  # 2048: quarter width per qc-row in the mask slab
    GW = 2 * QCH  # 1024: exp/TT group width (2 k-chunks, fp32 PSUM = 2 banks)

    with (
        tc.tile_pool(name="const", bufs=1) as constp,
        tc.tile_pool(name="qkslab", bufs=2) as qkp,
        tc.tile_pool(name="scratch", bufs=2, space="DRAM") as scrp,
        tc.tile_pool(name="vst", bufs=4) as vp,
        tc.tile_pool(name="maskp", bufs=6) as maskp,
        tc.tile_pool(name="ptp", bufs=8) as ptp,
        tc.tile_pool(name="osbp", bufs=10) as osbp,
        tc.tile_pool(name="ofp", bufs=4) as ofp,
        tc.tile_pool(name="rcp", bufs=4) as rcp,
        tc.tile_pool(name="ps_s", bufs=2, space="PSUM") as ps_s,
        tc.tile_pool(name="ps_po", bufs=2, space="PSUM") as ps_po,
        tc.tile_pool(name="ps_e", bufs=2, space="PSUM") as ps_e,
    ):
        identF = constp.tile([P, P], dt.float32)
        make_identity(nc, identF)
        # PE warmup through the initial DMA wait (HAM -> K=8/8).
        wsrc = constp.tile([P, QCH], dt.bfloat16)
        nc.vector.memset(wsrc[:], 0.0)
        wps = ps_s.tile([P, GW], dt.float32, tag="st")
        for _ in range(30):
            nc.tensor.matmul(
                wps[:, 0:QCH], wsrc[:, 0:P], wsrc[:], start=True, stop=True
            )

        # ---------------- per-pair load emitters ----------------
        slabs = {}  # pr -> (QT2, KT2)
        vsts = {}  # pr -> [vt_hi0, vt_hi1]
        msl = {}  # (pr, qt, hi) -> slab

        def emit_qkv(pr):
            heads = (2 * pr, 2 * pr + 1)
            sl = {}
            for name, src_ap in (("q", q_ap), ("k", k_ap)):
                scr = scrp.tile([S, P], dt.bfloat16, tag=f"scr_{name}")
                for hi, h in enumerate(heads):
                    nc.gpsimd.dma_start(scr[:, hi * D : (hi + 1) * D], src_ap[h])
                slab = qkp.tile([P, S], dt.bfloat16, tag=f"{name}t2")
                nc.sync.dma_start(slab[:], scr[:], transpose=True)
                sl[name] = slab
            slabs[pr] = (sl["q"], sl["k"])
            vv = []
            for hi, h in enumerate(heads):
                vt = vp.tile([P, n_kch * (D + 1)], dt.bfloat16, tag="vst")
                vt3 = vt[:].rearrange("p (t c) -> p t c", c=D + 1)
                nc.gpsimd.dma_start(
                    vt3[:, :, 0:D], v_ap[h].rearrange("(t p) d -> p t d", p=P)
                )
                nc.vector.memset(vt3[:, :, D : D + 1], 1.0)
                vv.append(vt)
            vsts[pr] = vv

        def emit_mask(pr, quarters):
            heads = (2 * pr, 2 * pr + 1)
            for qt in quarters:
                for hi, h in enumerate(heads):
                    ms = maskp.tile([P, n_qc * KLQ * QCH], dt.bfloat16, tag="ms")
                    src = m_ap[h, qt * KLQ * P : (qt + 1) * KLQ * P, :].rearrange(
                        "(kl p) (qc j) -> p qc kl j", p=P, j=QCH
                    )
                    dst = ms[:].rearrange(
                        "p (qc kl j) -> p qc kl j", qc=n_qc, kl=KLQ
                    )
                    nc.gpsimd.dma_start(dst, src)
                    msl[(pr, qt, hi)] = ms

        # pair 0: slabs first, then its early mask quarters
        emit_qkv(0)
        emit_mask(0, (0, 1))

        for pr in range(n_pairs):
            heads = (2 * pr, 2 * pr + 1)
            QT2, KT2 = slabs[pr]
            vst = vsts[pr]

            # prefetch: next pair's slabs/V go on the SWDGE queue BEFORE this
            # pair's late mask quarters (and before next pair's own mask).
            if pr + 1 < n_pairs:
                emit_qkv(pr + 1)
            emit_mask(pr, (2, 3))
            if pr + 1 < n_pairs:
                emit_mask(pr + 1, (0, 1))

            osb = {}
            for half in range(2):
                for qc in range(n_qc):
                    q0 = qc * QCH
                    po = [
                        ps_po.tile(
                            [D + 1, QCH], dt.float32, tag="po", name=f"po{hi_}"
                        )
                        for hi_ in range(2)
                    ]
                    for g in range(4):  # four 2-chunk groups per half
                        qt = half * 2 + g // 2  # mask quarter
                        klq = (2 * g) % KLQ  # first chunk's kl within quarter
                        sts = [
                            ps_s.tile(
                                [P, GW], dt.float32, tag="st", name=f"st{hi_}"
                            )
                            for hi_ in range(2)
                        ]
                        # QK: interleave heads so A/B matmuls are adjacent
                        # (disjoint row groups -> PE runs them concurrently)
                        for h2 in range(2):
                            ki = half * 8 + 2 * g + h2
                            for hi in range(2):
                                nc.tensor.matmul(
                                    sts[hi][:, h2 * QCH : (h2 + 1) * QCH],
                                    KT2[
                                        hi * D : (hi + 1) * D,
                                        ki * KCH : (ki + 1) * KCH,
                                    ],
                                    QT2[hi * D : (hi + 1) * D, q0 : q0 + QCH],
                                    start=True,
                                    stop=True,
                                )
                        pts = []
                        for hi in range(2):
                            pt = ptp.tile(
                                [P, GW], dt.bfloat16, tag="pt", name=f"pt{hi}"
                            )
                            nc.scalar.activation(
                                pt[:],
                                sts[hi][:],
                                mybir.ActivationFunctionType.Exp,
                                scale=SCALE,
                            )
                            ms = msl[(pr, qt, hi)]
                            off = qc * QW + klq * QCH
                            nc.vector.tensor_mul(
                                pt[:], pt[:], ms[:, off : off + GW]
                            )
                            pts.append(pt)
                        for hi in range(2):
                            for h2 in range(2):
                                ki = half * 8 + 2 * g + h2
                                nc.tensor.matmul(
                                    po[hi][:],
                                    vst[hi][:, ki * (D + 1) : (ki + 1) * (D + 1)],
                                    pts[hi][:, h2 * QCH : (h2 + 1) * QCH],
                                    start=(g == 0 and h2 == 0),
                                    stop=(g == 3 and h2 == 1),
                                    skip_group_check=True,
                                )
                    # combine halves in SBUF
                    for hi in range(2):
                        if half == 0:
                            ot_acc = osbp.tile([D + 1, QCH], dt.float32, tag="osb")
                            nc.vector.tensor_copy(ot_acc[:], po[hi][:])
                            osb[(qc, hi)] = ot_acc
                        else:
                            nc.vector.tensor_add(
                                osb[(qc, hi)][:], osb[(qc, hi)][:], po[hi][:]
                            )

                    if half == 0:
                        continue
                    # ---- epilogue: transpose, normalize, store ----
                    for hi, h in enumerate(heads):
                        acc = osb[(qc, hi)]
                        pst = ps_e.tile([P, 4 * (D + 1)], dt.float32, tag="pst")
                        for ot in range(4):
                            nc.tensor.transpose(
                                pst[:, ot * (D + 1) : (ot + 1) * (D + 1)],
                                acc[:, ot * P : (ot + 1) * P],
                                identF[0 : D + 1, 0 : D + 1],
                            )
                        rc = rcp.tile([P, 4], dt.float32, tag="rc")
                        nc.vector.reciprocal(
                            rc[:].rearrange("p (ot c) -> p ot c", c=1),
                            pst[:].rearrange("p (ot c) -> p ot c", c=D + 1)[
                                :, :, D : D + 1
                            ],
                        )
                        of = ofp.tile([P, 4 * D], dt.float32, tag="of")
                        for ot in range(4):
                            nc.vector.tensor_scalar_mul(
                                of[:, ot * D : (ot + 1) * D],
                                pst[:, ot * (D + 1) : ot * (D + 1) + D],
                                rc[:, ot : ot + 1],
                            )
                        nc.sync.dma_start(
                            o_ap[h, q0 : q0 + QCH, :].rearrange(
                                "(ot p) d -> p ot d", p=P
                            ),
                            of[:].rearrange("p (ot d) -> p ot d", d=D),
                        )


def build_program(H=H_PER_CORE, S=S_FULL, **flags):
    nc = bacc.Bacc()
    q = nc.dram_tensor("q", [H, S, D], mybir.dt.float32, kind="ExternalInput")
    k = nc.dram_tensor("k", [H, S, D], mybir.dt.float32, kind="ExternalInput")
    v = nc.dram_tensor("v", [H, S, D], mybir.dt.float32, kind="ExternalInput")
    m = nc.dram_tensor("m", [H, S, S], mybir.dt.uint8, kind="ExternalInput")
    o = nc.dram_tensor("o", [H, S, D], mybir.dt.float32, kind="ExternalOutput")
    with tile.TileContext(nc) as tc:
        build_attention(tc, o.ap(), q.ap(), k.ap(), v.ap(), m.ap(), H=H, S=S, **flags)
    nc.compile()
    return nc


_CACHE = {}
LAST_RESULTS = None


def kernel(queries, keys, values, mask):
    global LAST_RESULTS
    if "nc" not in _CACHE:
        _CACHE["nc"] = build_program()
    nc = _CACHE["nc"]

    queries = np.ascontiguousarray(queries, dtype=np.float32)
    keys = np.ascontiguousarray(keys, dtype=np.float32)
    values = np.ascontiguousarray(values, dtype=np.float32)
    # ship the KEEP mask (1 = keep) transposed ([BH, k, q]), u8
    keep_u8 = np.ascontiguousarray(
        (~np.asarray(mask)).transpose(0, 2, 1)
    ).view(np.uint8)

    in_maps = []
    for c in range(N_CORES):
        sl = slice(c * H_PER_CORE, (c + 1) * H_PER_CORE)
        in_maps.append(
            {
                "q": queries[sl],
                "k": keys[sl],
                "v": values[sl],
                "m": keep_u8[sl],
            }
        )

    trace = bool(int(os.environ.get("ATTN_TRACE", "0")))
    res = run_bass_kernel_spmd(
        nc, in_maps, core_ids=list(range(N_CORES)), trace=trace
    )
    LAST_RESULTS = res
    return np.concatenate([r["o"] for r in res.results], axis=0)


# revision 13
# speedup vs baseline: 1.1466x; 1.1466x over previous
"""Masked multi-head attention kernel for Trainium2 (Bass/Tile), 8-core SPMD.

Problem: BH=64 heads of S=2048, D=64 attention with a dense bool mask,
scale = 1/sqrt(1024).  Sharded 8 heads per NeuronCore (no cross-core comm).

Per-core dataflow (heads processed in pairs; ACT-exp paces the kernel):
  - Q,K: SWDGE cast-DMA f32->bf16 HBM->HBM into a pair-interleaved scratch
    [S, 128] (head A cols 0:64, head B cols 64:128), then one HWDGE xbar
    transpose-DMA per (pair, tensor) builds the [d2, S] slab in SBUF.
  - V: SWDGE cast-DMA into [128, 16*65] chunk-major tiles; col 64 of each
    65-group memset to 1.0 (gives softmax denominators via the AV matmul).
  - mask: host sends KEEP mask (1=keep) transposed [H, S_k, S_q] u8; SWDGE
    cast-DMA u8->bf16 into quarter slabs laid out [p, qc, kl, j] so the
    multiply operand per (qc, 4-chunk group) is one contiguous [128, 2048].
  - S^T = K Q^T on the PE with the two heads' matmuls issued adjacently
    (disjoint 64-row groups -> concurrent); scores land in PSUM as bf16 so
    a [128, 2048] tile is 2 banks and exp runs at FD=2048.
  - exp on ACT (scale=1/32) PSUM->SBUF bf16; mask applied post-exp as one
    DVE tensor_mul [128, 2048] (bf16 2x mode) giving exact zeros.
  - AV: stationary [V|1] (M=65) streams masked P^T, accumulating the 8
    k-chunks of a half in PSUM; halves combined with one DVE add.
  - Epilogue per (qc, head): 4 PE transposes into one packed PSUM tile,
    one strided reciprocal, 4 per-partition scales, one batched store.
  - Cross-pair prefetch: next pair's slab/V DMAs are emitted between the
    current pair's early and late mask quarters so they are not FIFO-stuck
    behind 16 MB of mask traffic on the single SWDGE queue.
"""

import os
import sys

sys.path.insert(0, "/opt/trn_rl_repo")

import numpy as np

import concourse.bass as bass
import concourse.mybir as mybir
import concourse.tile as tile
from concourse import bacc
from concourse.bass_utils import run_bass_kernel_spmd
from concourse.masks import make_identity

N_CORES = 8
BH, S_FULL, D = 64, 2048, 64
H_PER_CORE = BH // N_CORES  # 8
P = 128  # SBUF/PSUM partitions
KCH = 128  # k-chunk rows
QCH = 512  # q-chunk cols
SCALE = 1.0 / 32.0  # 1/sqrt(1024) per the module spec


def build_attention(tc, o_ap, q_ap, k_ap, v_ap, m_ap, H, S):
    nc = tc.nc
    dt = mybir.dt
    n_pairs = H // 2
    n_kch = S // KCH  # 16 k-chunks per head
    n_qc = S // QCH  # 4 q-chunks
    n_quart = 4
    KLQ = n_kch // n_quart  # 4 k-chunks per mask quarter
    QW = KLQ * QCH  # 2048: quarter width per qc-row in the mask slab
    GW = 2 * QCH  # 1024: exp/TT group width (2 k-chunks, fp32 PSUM = 2 banks)

    with (
        tc.tile_pool(name="const", bufs=1) as constp,
        tc.tile_pool(name="qkslab", bufs=2) as qkp,
        tc.tile_pool(name="scratch", bufs=2, space="DRAM") as scrp,
        tc.tile_pool(name="vst", bufs=4) as vp,
        tc.tile_pool(name="maskp", bufs=6) as maskp,
        tc.tile_pool(name="ptp", bufs=8) as ptp,
        tc.tile_pool(name="osbp", bufs=10) as osbp,
        tc.tile_pool(name="ofp", bufs=4) as ofp,
        tc.tile_pool(name="rcp", bufs=4) as rcp,
        tc.tile_pool(name="ps_s", bufs=2, space="PSUM") as ps_s,
        tc.tile_pool(name="ps_po", bufs=2, space="PSUM") as ps_po,
        tc.tile_pool(name="ps_e", bufs=2, space="PSUM") as ps_e,
    ):
        identF = constp.tile([P, P], dt.float32)
        make_identity(nc, identF)
        # PE warmup through the initial DMA wait (HAM -> K=8/8).
        wsrc = constp.tile([P, QCH], dt.bfloat16)
        nc.vector.memset(wsrc[:], 0.0)
        wps = ps_s.tile([P, GW], dt.float32, tag="st")
        for _ in range(30):
            nc.tensor.matmul(
                wps[:, 0:QCH], wsrc[:, 0:P], wsrc[:], start=True, stop=True
            )

        # ---------------- per-pair load emitters ----------------
        slabs = {}  # pr -> (QT2, KT2)
        vsts = {}  # pr -> [vt_hi0, vt_hi1]
        msl = {}  # (pr, qt, hi) -> slab

        def emit_scr(pr, name, src_ap):
            heads = (2 * pr, 2 * pr + 1)
            scr = scrp.tile([S, P], dt.bfloat16, tag=f"scr_{name}", name=f"scr{name}{pr}")
            for hi, h in enumerate(heads):
                nc.gpsimd.dma_start(scr[:, hi * D : (hi + 1) * D], src_ap[h])
            slab = qkp.tile([P, S], dt.bfloat16, tag=f"{name}t2", name=f"{name}t2_{pr}")
            nc.sync.dma_start(slab[:], scr[:], transpose=True)
            return slab

        def emit_v(pr):
            heads = (2 * pr, 2 * pr + 1)
            vv = []
            for hi, h in enumerate(heads):
                vt = vp.tile([P, n_kch * (D + 1)], dt.bfloat16, tag="vst")
                vt3 = vt[:].rearrange("p (t c) -> p t c", c=D + 1)
                nc.gpsimd.dma_start(
                    vt3[:, :, 0:D], v_ap[h].rearrange("(t p) d -> p t d", p=P)
                )
                nc.vector.memset(vt3[:, :, D : D + 1], 1.0)
                vv.append(vt)
            vsts[pr] = vv

        def emit_qkv(pr):
            slabs[pr] = (emit_scr(pr, "q", q_ap), emit_scr(pr, "k", k_ap))
            emit_v(pr)

        def emit_mask(pr, quarters, his=(0, 1)):
            heads = (2 * pr, 2 * pr + 1)
            for qt in quarters:
                for hi in his:
                    h = heads[hi]
                    ms = maskp.tile([P, n_qc * KLQ * QCH], dt.bfloat16, tag="ms")
                    src = m_ap[h, qt * KLQ * P : (qt + 1) * KLQ * P, :].rearrange(
                        "(kl p) (qc j) -> p qc kl j", p=P, j=QCH
                    )
                    dst = ms[:].rearrange(
                        "p (qc kl j) -> p qc kl j", qc=n_qc, kl=KLQ
                    )
                    nc.gpsimd.dma_start(dst, src)
                    msl[(pr, qt, hi)] = ms

        # pair 0: interleave slab and mask loads on the SWDGE FIFO so the
        # first compute unit (needs slabs + head-0's first mask quarter) is
        # ready as early as possible.
        q0slab = emit_scr(0, "q", q_ap)
        emit_mask(0, (0,), his=(0,))
        k0slab = emit_scr(0, "k", k_ap)
        emit_mask(0, (0,), his=(1,))
        slabs[0] = (q0slab, k0slab)
        emit_v(0)
        emit_mask(0, (1,))

        def make_epilogue(heads, q0, acc_pair):
            def emit():
                for hi, h in enumerate(heads):
                    acc = acc_pair[hi]
                    pst = ps_e.tile([P, 4 * (D + 1)], dt.float32, tag="pst")
                    for ot in range(4):
                        nc.tensor.transpose(
                            pst[:, ot * (D + 1) : (ot + 1) * (D + 1)],
                            acc[:, ot * P : (ot + 1) * P],
                            identF[0 : D + 1, 0 : D + 1],
                        )
                    rc = rcp.tile([P, 4], dt.float32, tag="rc")
                    nc.vector.reciprocal(
                        rc[:].rearrange("p (ot c) -> p ot c", c=1),
                        pst[:].rearrange("p (ot c) -> p ot c", c=D + 1)[
                            :, :, D : D + 1
                        ],
                    )
                    of = ofp.tile([P, 4 * D], dt.float32, tag="of")
                    for ot in range(4):
                        nc.vector.tensor_scalar_mul(
                            of[:, ot * D : (ot + 1) * D],
                            pst[:, ot * (D + 1) : ot * (D + 1) + D],
                            rc[:, ot : ot + 1],
                        )
                    nc.sync.dma_start(
                        o_ap[h, q0 : q0 + QCH, :].rearrange(
                            "(ot p) d -> p ot d", p=P
                        ),
                        of[:].rearrange("p (ot d) -> p ot d", d=D),
                    )

            return emit

        pending = []  # deferred epilogue emitters

        for pr in range(n_pairs):
            heads = (2 * pr, 2 * pr + 1)
            QT2, KT2 = slabs[pr]
            vst = vsts[pr]

            # prefetch: next pair's slabs/V go on the SWDGE queue BEFORE this
            # pair's late mask quarters (and before next pair's own mask).
            if pr + 1 < n_pairs:
                emit_qkv(pr + 1)
            emit_mask(pr, (2, 3))
            if pr + 1 < n_pairs:
                emit_mask(pr + 1, (0, 1))

            osb = {}
            for half in range(2):
                for qc in range(n_qc):
                    q0 = qc * QCH
                    po = [
                        ps_po.tile(
                            [D + 1, QCH], dt.float32, tag="po", name=f"po{hi_}"
                        )
                        for hi_ in range(2)
                    ]
                    for kg in range(4):  # four 2-chunk groups per half
                        qt = half * 2 + kg // 2  # mask quarter
                        klq = (2 * kg) % KLQ  # first chunk's kl within quarter
                        for hi in range(2):
                            st = ps_s.tile([P, GW], dt.float32, tag="st")
                            for h2 in range(2):
                                ki = half * 8 + 2 * kg + h2
                                nc.tensor.matmul(
                                    st[:, h2 * QCH : (h2 + 1) * QCH],
                                    KT2[
                                        hi * D : (hi + 1) * D,
                                        ki * KCH : (ki + 1) * KCH,
                                    ],
                                    QT2[hi * D : (hi + 1) * D, q0 : q0 + QCH],
                                    start=True,
                                    stop=True,
                                )
                            pt = ptp.tile([P, GW], dt.bfloat16, tag="pt")
                            nc.scalar.activation(
                                pt[:],
                                st[:],
                                mybir.ActivationFunctionType.Exp,
                                scale=SCALE,
                            )
                            ms = msl[(pr, qt, hi)]
                            off = qc * QW + klq * QCH
                            nc.vector.tensor_mul(
                                pt[:], pt[:], ms[:, off : off + GW]
                            )
                            for h2 in range(2):
                                ki = half * 8 + 2 * kg + h2
                                nc.tensor.matmul(
                                    po[hi][:],
                                    vst[hi][:, ki * (D + 1) : (ki + 1) * (D + 1)],
                                    pt[:, h2 * QCH : (h2 + 1) * QCH],
                                    start=(kg == 0 and h2 == 0),
                                    stop=(kg == 3 and h2 == 1),
                                    skip_group_check=True,
                                )
                        # emit a deferred epilogue between groups so its PE
                        # transposes interleave with this block's matmuls
                        # instead of blocking the pipeline at a qc boundary
                        if kg == 1 and pending:
                            pending.pop(0)()
                    # combine halves in SBUF
                    for hi in range(2):
                        if half == 0:
                            ot_acc = osbp.tile([D + 1, QCH], dt.float32, tag="osb")
                            nc.vector.tensor_copy(ot_acc[:], po[hi][:])
                            osb[(qc, hi)] = ot_acc
                        else:
                            nc.vector.tensor_add(
                                osb[(qc, hi)][:], osb[(qc, hi)][:], po[hi][:]
                            )
                    if half == 1:
                        pending.append(
                            make_epilogue(
                                heads, q0, (osb[(qc, 0)], osb[(qc, 1)])
                            )
                        )

        for emit in pending:
            emit()


def build_program(H=H_PER_CORE, S=S_FULL, **flags):
    nc = bacc.Bacc()
    q = nc.dram_tensor("q", [H, S, D], mybir.dt.float32, kind="ExternalInput")
    k = nc.dram_tensor("k", [H, S, D], mybir.dt.float32, kind="ExternalInput")
    v = nc.dram_tensor("v", [H, S, D], mybir.dt.float32, kind="ExternalInput")
    m = nc.dram_tensor("m", [H, S, S], mybir.dt.uint8, kind="ExternalInput")
    o = nc.dram_tensor("o", [H, S, D], mybir.dt.float32, kind="ExternalOutput")
    with tile.TileContext(nc) as tc:
        build_attention(tc, o.ap(), q.ap(), k.ap(), v.ap(), m.ap(), H=H, S=S, **flags)
    nc.compile()
    return nc


_CACHE = {}
LAST_RESULTS = None


def kernel(queries, keys, values, mask):
    global LAST_RESULTS
    if "nc" not in _CACHE:
        _CACHE["nc"] = build_program()
    nc = _CACHE["nc"]

    queries = np.ascontiguousarray(queries, dtype=np.float32)
    keys = np.ascontiguousarray(keys, dtype=np.float32)
    values = np.ascontiguousarray(values, dtype=np.float32)
    # ship the KEEP mask (1 = keep) transposed ([BH, k, q]), u8
    keep_u8 = np.ascontiguousarray(
        (~np.asarray(mask)).transpose(0, 2, 1)
    ).view(np.uint8)

    in_maps = []
    for c in range(N_CORES):
        sl = slice(c * H_PER_CORE, (c + 1) * H_PER_CORE)
        in_maps.append(
            {
                "q": queries[sl],
                "k": keys[sl],
                "v": values[sl],
                "m": keep_u8[sl],
            }
        )

    trace = bool(int(os.environ.get("ATTN_TRACE", "0")))
    res = run_bass_kernel_spmd(
        nc, in_maps, core_ids=list(range(N_CORES)), trace=trace
    )
    LAST_RESULTS = res
    return np.concatenate([r["o"] for r in res.results], axis=0)
